# revision 43
# baseline (speedup 1.0000x reference)
"""MoE kernel for 8 TRN2 NeuronCores.

Strategy (expert-parallel, routing-as-sharding):
  - Router (Linear-GELU-Linear-softmax-top2) runs on host in f64 numpy;
    verified to reproduce the jax f32 reference top-2 sets exactly.
  - Token tiles (128 tokens, single expert each) are bin-packed onto the
    8 cores in up to two uniform "segments" per core: segment A runs sA
    tiles with one expert's weights, segment B runs sB tiles with a second
    expert's weights (loaded mid-kernel, overlapped with compute).
  - Per-core Bass kernel: 3-layer expert MLP with LayerNorm+exact-GELU
    between layers, bf16 matmuls with f32 PSUM accumulation, LN stats read
    PSUM directly, combine-weight scaling fused into output eviction.
    Software-pipelined across tiles (3-stage skew) to keep the PE busy.
  - LN rstd is a quake-style inverse sqrt on the vector engine (seed via
    exponent bit-hack + Newton), so the scalar engine only ever runs GELU
    and its activation table loads exactly once.
  - Weight/token DMAs are issued in consumption order (c-strips) across
    the idle queues; dummy identity matmuls warm the PE clock (HAM) while
    the first weights stream in.
  - Host scatter-adds the two expert contributions per token.
"""

import math
import os

import numpy as np

D, H, E, K = 512, 2048, 8, 2
EPS = 1e-5
P = 128
# quake rsqrt magic, pre-adjusted so the seed read from bits of hv=ve/2
# approximates ve^-1/2 (0x5f3759df - 0x00400000)
QUAKE_MAGIC = 0x5F3759DF - 0x00400000
NR_ITERS = 1

last_exec_time_ns = None


def _gelu_exact(x):
    from scipy.special import erf

    return 0.5 * x * (1.0 + erf(x / np.sqrt(2.0)))


def _route(t, Wg1, bg1, Wg2, bg2):
    th = t.astype(np.float64)
    h = th @ Wg1.astype(np.float64) + bg1.astype(np.float64)
    h = _gelu_exact(h)
    logits = h @ Wg2.astype(np.float64) + bg2.astype(np.float64)
    logits = logits - logits.max(axis=-1, keepdims=True)
    ex = np.exp(logits)
    gates = ex / ex.sum(axis=-1, keepdims=True)
    top2 = np.argsort(-gates, axis=-1, kind="stable")[:, :K]
    topv = np.take_along_axis(gates, top2, axis=-1)
    topv = topv / topv.sum(axis=-1, keepdims=True)
    return top2, topv.astype(np.float32)


def _pack_segments(tiles, n_slots=8):
    """Find minimal S and split S = sA + sB such that every expert's tile
    count can be covered by a_e A-slots (sA tiles each) + b_e B-slots (sB
    tiles each) with sum(a) <= n_slots, sum(b) <= n_slots.

    Returns (sA, sB, assign) where assign[e] = (a_e, b_e)."""
    total = sum(tiles)
    s_lo = max(1, (total + n_slots - 1) // n_slots)
    for S in range(s_lo, max(tiles) + 1):
        for sA in range(S, (S - 1) // 2, -1):
            sB = S - sA
            states = {(0, 0): 0}
            back = []
            ok = True
            for t in tiles:
                opts = []
                for a in range(n_slots + 1):
                    for b in range(n_slots + 1):
                        cap = a * sA + b * sB
                        if cap >= t:
                            opts.append((a, b, cap - t))
                new = {}
                for (au, bu), w in states.items():
                    for a, b, waste in opts:
                        if au + a <= n_slots and bu + b <= n_slots:
                            key = (au + a, bu + b)
                            val = (w + waste, (au, bu), (a, b))
                            if key not in new or new[key][0] > val[0]:
                                new[key] = val
                if not new:
                    ok = False
                    break
                back.append(new)
                states = {k: v[0] for k, v in new.items()}
            if not ok:
                continue
            key = min(states, key=lambda k: states[k])
            assign = []
            for st in reversed(back):
                w, prev, ab = st[key]
                assign.append(ab)
                key = prev
            return sA, sB, list(reversed(assign))
    return max(tiles), 0, [(1, 0)] * len(tiles)


def _build_program(n_a, n_b, affine, compute_dt_name="bfloat16"):
    """Per-core Bass program: n_a tiles with weight-set A, then n_b tiles
    with weight-set B (B weights streamed in mid-kernel)."""
    from concourse import bacc, bass, tile, mybir
    from concourse import masks

    f32 = mybir.dt.float32
    u32 = mybir.dt.uint32
    bf16 = getattr(mybir.dt, compute_dt_name)
    AF = mybir.ActivationFunctionType
    ALU = mybir.AluOpType

    n_tiles = n_a + n_b
    C = n_tiles * P
    two_seg = n_b > 0
    KD = D // P
    KH = H // P
    KQ = KH // 4
    CS = 512

    nc = bacc.Bacc(None, target_bir_lowering=False, debug=False)

    # host-packed, partition-major pieces chosen so BOTH the DRAM source and
    # the SBUF destination of every DMA are fully contiguous per partition:
    # strided SBUF writes drop the per-queue rate from ~300GB/s to ~100GB/s,
    # and descriptor generation cost scales with row count. Weight strips
    # therefore live in per-c-strip SBUF tiles.
    NF = min(6, n_tiles)  # prologue depth
    F3 = NF * P
    tT0_d = nc.dram_tensor("tT0", (P, KD, P), bf16, kind="ExternalInput")
    if NF > 1:
        tTm_d = nc.dram_tensor("tTm", (P, KD, F3 - P), bf16, kind="ExternalInput")
    if F3 < C:
        tTr_d = nc.dram_tensor("tTr", (P, KD, C - F3), bf16, kind="ExternalInput")
    w1a_d = nc.dram_tensor("W1a", (4, P, KD, CS), bf16, kind="ExternalInput")
    w2a_d = nc.dram_tensor("W2a", (4, P, KH, CS), bf16, kind="ExternalInput")
    w3a_d = nc.dram_tensor("W3a", (P, KH, D), bf16, kind="ExternalInput")
    cw_d = nc.dram_tensor("cw", (P, n_tiles), f32, kind="ExternalInput")
    out_d = nc.dram_tensor("out", (C, D), f32, kind="ExternalOutput")
    if two_seg:
        w1b_d = nc.dram_tensor("W1b", (P, KD, H), bf16, kind="ExternalInput")
        w2b_d = nc.dram_tensor("W2b", (4, P, KH, CS), bf16, kind="ExternalInput")
        w3b_d = nc.dram_tensor("W3b", (P, KH, D), bf16, kind="ExternalInput")

    aff_d = {}
    for name, width in (
        ("b1", H), ("g1", H), ("be1", H),
        ("b2", H), ("g2", H), ("be2", H),
        ("b3", D),
    ):
        if affine[name]:
            aff_d[name] = nc.dram_tensor(name, (P, width), f32, kind="ExternalInput")

    with tile.TileContext(nc) as tc:
        with (
            tc.tile_pool(name="const", bufs=1) as const_pool,
            tc.tile_pool(name="hraw", bufs=2) as hraw_pool,
            tc.tile_pool(name="xg1", bufs=7) as xg1_pool,
            tc.tile_pool(name="xg2", bufs=2) as xg2_pool,
            tc.tile_pool(name="hT", bufs=2) as hT_pool,
            tc.tile_pool(name="outp", bufs=2) as out_pool,
            tc.tile_pool(name="st", bufs=8) as st_pool,
            tc.tile_pool(name="acc", bufs=6, space="PSUM") as acc_pool,
            tc.tile_pool(name="tp", bufs=2, space="PSUM") as tp_pool,
        ):
            # ---- resident tiles (weights per c-strip for contiguous DMA) --
            w1a_c = [const_pool.tile((P, KD, CS), bf16, name=f"w1a{c}")
                     for c in range(4)]
            w2a_c = [const_pool.tile((P, KH, CS), bf16, name=f"w2a{c}")
                     for c in range(4)]
            w3a_s = const_pool.tile((P, KH, D), bf16)
            tT0_s = const_pool.tile((P, KD, P), bf16)
            if NF > 1:
                tTm_s = const_pool.tile((P, KD, F3 - P), bf16)
            if F3 < C:
                tTr_s = const_pool.tile((P, KD, C - F3), bf16)
            cw_s = const_pool.tile((P, n_tiles), f32)
            if two_seg:
                w1b_s = const_pool.tile((P, KD, H), bf16)
                w3b_s = const_pool.tile((P, KH, D), bf16)

            # ---- PE clock warm-up prep: a dep-free bf16 operand tile so
            # dummy matmuls can start the HAM un-throttle (~3.4us busy)
            # as soon as the engine preamble ends
            dummy_bf = const_pool.tile((P, P), bf16, name="dummy_bf")
            nc.vector.memset(dummy_bf[:], 1.0)
            warm_ps = acc_pool.tile((P, P), f32, name="warm_ps", tag="ps_acc")
            for _ in range(24):
                nc.tensor.matmul(warm_ps[:], dummy_bf[:], dummy_bf[:],
                                 start=True, stop=True)

            # ---- resident loads: one DMA per contiguous piece, each
            # queue's FIFO ordered by consumption deadline; per-queue
            # sustained rate is only ~140GB/s, so the three queues carry
            # ~4.5MB each and the PD-deep L1 prologue hides the stream ----
            # sync: tile-0 tokens, W1a c1, prologue tokens, W2a even quads
            nc.sync.dma_start(tT0_s[:], tT0_d[:])
            nc.sync.dma_start(w1a_c[1][:], w1a_d[1])
            if NF > 1:
                nc.sync.dma_start(tTm_s[:], tTm_d[:])
            for c in range(4):
                for q in (0, 2):
                    nc.sync.dma_start(
                        w2a_c[c][:, 4 * q:4 * q + 4, :], w2a_d[c, :, 4 * q:4 * q + 4, :]
                    )
            nc.sync.dma_start(cw_s[:], cw_d[:])
            # scalar: W1a c0/c3, W2a odd quads
            nc.scalar.dma_start(w1a_c[0][:], w1a_d[0])
            nc.scalar.dma_start(w1a_c[3][:], w1a_d[3])
            for c in range(4):
                for q in (1, 3):
                    nc.scalar.dma_start(
                        w2a_c[c][:, 4 * q:4 * q + 4, :], w2a_d[c, :, 4 * q:4 * q + 4, :]
                    )
            # gpsimd: W1a c2, token rest, W3a halves
            nc.gpsimd.dma_start(w1a_c[2][:], w1a_d[2])
            if F3 < C:
                nc.gpsimd.dma_start(tTr_s[:], tTr_d[:])
            nc.gpsimd.dma_start(w3a_s[:, 0:8, :], w3a_d[:, 0:8, :])
            nc.gpsimd.dma_start(w3a_s[:, 8:16, :], w3a_d[:, 8:16, :])
            # segment-B W1/W3 at the tail of the lighter queues: must be
            # emitted BEFORE any stage_a(i>=n_a) so the RAW dep exists
            if two_seg:
                nc.gpsimd.dma_start(w1b_s[:, :, :], w1b_d[:])
                nc.scalar.dma_start(w3b_s[:, :, :], w3b_d[:])

            # ---- small constants (after the DMA issues so their engine
            # work does not delay descriptor generation) ----
            identity = const_pool.tile((P, P), bf16)
            masks.make_identity(nc, identity[:])
            magic_t = const_pool.tile((P, 1), u32, name="magic_t")
            nc.vector.memset(magic_t[:], QUAKE_MAGIC)
            c15_t = const_pool.tile((P, 1), f32, name="c15_t")
            nc.vector.memset(c15_t[:], 1.5)
            warm_t = const_pool.tile((P, 1), f32, name="warm_t")
            nc.vector.memset(warm_t[:], 0.0)
            # preload the GELU activation table while DMAs stream (bias/
            # scale APs match the real evictions' ACT variant)
            nc.scalar.activation(warm_t[:], warm_t[:], AF.Gelu,
                                 bias=c15_t[:], scale=c15_t[:])

            aff_s = {}
            for name in aff_d:
                width = aff_d[name].shape[1]
                row = const_pool.tile((P, width), f32, name=f"{name}_bcast")
                nc.sync.dma_start(row[:], aff_d[name][:])
                aff_s[name] = row

            def w1a_get(k, c):
                return w1a_c[c][:, k, :]

            def w1b_get(k, c):
                return w1b_s[:, k, c * CS:(c + 1) * CS]

            def w2_get(k, c):
                return w2a_c[c][:, k, :]

            def weights_for(i):
                if (not two_seg) or i < n_a:
                    return w1a_get, w2_get, w3a_s
                return w1b_get, w2_get, w3b_s

            def tok_lhsT(i, k):
                if i == 0:
                    return tT0_s[:, k, :]
                if i < NF:
                    return tTm_s[:, k, (i - 1) * P:i * P]
                return tTr_s[:, k, (i - NF) * P:(i - NF + 1) * P]

            def quake_rstd_negmr(mv):
                """rstd = (var+eps)^-1/2 and negmr = -mean*rstd, DVE only."""
                hv = st_pool.tile((P, 1), f32, name="hv", tag="hv")
                nc.vector.tensor_scalar(
                    out=hv[:], in0=mv[:, 1:2], scalar1=float(EPS),
                    scalar2=0.5, op0=ALU.add, op1=ALU.mult,
                )
                ysh = st_pool.tile((P, 1), f32, name="ysh", tag="ysh")
                nc.vector.tensor_scalar(
                    out=ysh[:].bitcast(u32), in0=hv[:].bitcast(u32),
                    scalar1=1, scalar2=None, op0=ALU.arith_shift_right,
                )
                y = st_pool.tile((P, 1), f32, name="yq", tag="yq")
                nc.vector.tensor_tensor(
                    out=y[:].bitcast(u32), in0=magic_t[:],
                    in1=ysh[:].bitcast(u32), op=ALU.subtract,
                )
                for _ in range(NR_ITERS):
                    a = st_pool.tile((P, 1), f32, name="aq", tag="aq")
                    nc.vector.tensor_tensor(
                        out=a[:], in0=y[:], in1=y[:], op=ALU.mult,
                    )
                    cq = st_pool.tile((P, 1), f32, name="cq", tag="cq")
                    nc.vector.scalar_tensor_tensor(
                        out=cq[:], in0=a[:], scalar=hv[:], in1=c15_t[:],
                        op0=ALU.mult, op1=ALU.subtract,
                    )
                    y2 = st_pool.tile((P, 1), f32, name="y2q", tag="y2q")
                    nc.vector.tensor_scalar(
                        out=y2[:], in0=cq[:], scalar1=y[:], scalar2=-1.0,
                        op0=ALU.mult, op1=ALU.mult,
                    )
                    y = y2
                negmr = st_pool.tile((P, 1), f32, name="negmr", tag="negmr")
                nc.vector.tensor_scalar(
                    out=negmr[:], in0=mv[:, 0:1], scalar1=y[:], scalar2=-1.0,
                    op0=ALU.mult, op1=ALU.mult,
                )
                return y, negmr

            def mm_ln_gelu(tile_i, lhsT_getter, n_k, w_get, nh, bname, gname, bename, xg_tag):
                """matmul (-> +b) -> LN -> (*g +be) -> gelu; returns xg tile."""
                nch = nh // CS
                fast = not (affine[bname] or affine[gname] or affine[bename])
                hraw = None
                if not fast:
                    hraw = hraw_pool.tile((P, nh), f32, tag="hraw")
                stats = st_pool.tile((P, nch, 6), f32, tag="stats")
                ps_list = []
                for c in range(nch):
                    ps = acc_pool.tile((P, CS), f32, name="ps_acc", tag="ps_acc")
                    for k in range(n_k):
                        nc.tensor.matmul(
                            ps[:],
                            lhsT_getter(k),
                            w_get(k, c),
                            start=(k == 0),
                            stop=(k == n_k - 1),
                        )
                    cs_sl = slice(c * CS, (c + 1) * CS)
                    if fast:
                        nc.vector.bn_stats(stats[:, c, :], ps[:])
                        ps_list.append(ps)
                    else:
                        nc.scalar.copy(hraw[:, cs_sl], ps[:])
                        if affine[bname]:
                            nc.vector.tensor_tensor(
                                out=hraw[:, cs_sl], in0=hraw[:, cs_sl],
                                in1=aff_s[bname][:, cs_sl], op=ALU.add,
                            )
                        nc.vector.bn_stats(stats[:, c, :], hraw[:, cs_sl])
                mv = st_pool.tile((P, 2), f32, tag="mv")
                nc.vector.bn_aggr(mv[:], stats[:])
                rstd, negmr = quake_rstd_negmr(mv)
                pool = xg1_pool if xg_tag == "xg1" else xg2_pool
                xg = pool.tile((P, nh), bf16, tag=xg_tag)
                for c in range(nch):
                    cs_sl = slice(c * CS, (c + 1) * CS)
                    if fast:
                        nc.scalar.activation(
                            xg[:, cs_sl], ps_list[c][:], AF.Gelu,
                            bias=negmr[:], scale=rstd[:],
                        )
                    else:
                        xn = hraw_pool.tile((P, CS), f32, name="xn", tag="xn")
                        nc.vector.tensor_scalar(
                            out=xn[:], in0=hraw[:, cs_sl],
                            scalar1=mv[:, 0:1], scalar2=rstd[:],
                            op0=ALU.subtract, op1=ALU.mult,
                        )
                        if affine[gname]:
                            nc.vector.tensor_tensor(
                                out=xn[:], in0=xn[:], in1=aff_s[gname][:, cs_sl],
                                op=ALU.mult,
                            )
                        if affine[bename]:
                            nc.vector.tensor_tensor(
                                out=xn[:], in0=xn[:], in1=aff_s[bename][:, cs_sl],
                                op=ALU.add,
                            )
                        nc.scalar.activation(xg[:, cs_sl], xn[:], AF.Gelu)
                return xg

            def transpose_to_hT(xg, nh, hT_tag):
                """PE-transpose (P, nh) bf16 -> (P, nh//P, P) feature-major.
                Batches 8 blocks per PSUM tile (1 full bank) so the DVE
                evacuation keeps pace with the PE transposes."""
                TB = 2 * CS  # 1024 cols = 8 blocks per psum tile
                nbt = nh // TB
                hT = hT_pool.tile((P, nh // P, P), bf16, tag=hT_tag)
                for half in range(nbt):
                    pt = tp_pool.tile((P, TB), bf16, name="pt", tag="pt")
                    for j in range(TB // P):
                        b = half * (TB // P) + j
                        nc.tensor.transpose(
                            pt[:, j * P:(j + 1) * P],
                            xg[:, b * P:(b + 1) * P],
                            identity[:],
                        )
                    nc.vector.tensor_copy(
                        hT[:, half * (TB // P):(half + 1) * (TB // P), :], pt[:]
                    )
                return hT

            xg1 = {}
            xg2 = {}

            def stage_a(i):
                w1_get = weights_for(i)[0]
                xg1[i] = mm_ln_gelu(
                    i, lambda k: tok_lhsT(i, k), KD, w1_get, H,
                    "b1", "g1", "be1", "xg1",
                )

            def stage_b(i):
                w2_get = weights_for(i)[1]
                h1T = transpose_to_hT(xg1.pop(i), H, "hT1")
                xg2[i] = mm_ln_gelu(
                    i, lambda k: h1T[:, k, :], KH, w2_get, H,
                    "b2", "g2", "be2", "xg2",
                )

            def stage_c(i):
                w3_s = weights_for(i)[2]
                h2T = transpose_to_hT(xg2.pop(i), H, "hT2")
                ps3 = acc_pool.tile((P, D), f32, name="ps3", tag="ps_acc")
                for k in range(KH):
                    nc.tensor.matmul(
                        ps3[:], h2T[:, k, :], w3_s[:, k, :],
                        start=(k == 0), stop=(k == KH - 1),
                    )
                outt = out_pool.tile((P, D), f32, tag="outt")
                if affine["b3"]:
                    nc.vector.tensor_tensor(
                        out=outt[:], in0=ps3[:], in1=aff_s["b3"][:], op=ALU.add,
                    )
                    nc.scalar.mul(outt[:], outt[:], cw_s[:, i:i + 1])
                else:
                    nc.vector.tensor_scalar(
                        out=outt[:], in0=ps3[:],
                        scalar1=cw_s[:, i:i + 1], scalar2=None,
                        op0=ALU.mult, op1=ALU.bypass,
                    )
                tok = slice(i * P, (i + 1) * P)
                nc.gpsimd.dma_start(out_d[tok, :], outt[:])

            PD = NF
            for i in range(PD):
                stage_a(i)
            for j in range(n_tiles):
                if j + PD < n_tiles:
                    stage_a(j + PD)
                stage_b(j)
                # after the last segment-A consumer of W2 is issued,
                # stream segment-B W2 into the same SBUF tile (c-strip
                # order so L2 of tile n_a can start on strip c0; the WAR
                # dep on each strip region clears as soon as its last
                # segment-A reader retires)
                if two_seg and j == n_a - 1:
                    for c in range(4):
                        nc.sync.dma_start(w2a_c[c][:, 0:8, :], w2b_d[c, :, 0:8, :])
                        nc.scalar.dma_start(w2a_c[c][:, 8:16, :], w2b_d[c, :, 8:16, :])
                if j >= 1:
                    stage_c(j - 1)
            stage_c(n_tiles - 1)

    nc.compile()
    return nc


def kernel(**inputs):
    global last_exec_time_ns
    import ml_dtypes

    from concourse import bass_utils

    inp = {k: np.asarray(v) for k, v in inputs.items()}
    x = inp["x"].astype(np.float32, copy=False)
    B, S, d = x.shape
    T = B * S
    t = x.reshape(T, d)

    top2, topv = _route(t, inp["Wg1"], inp["bg1"], inp["Wg2"], inp["bg2"])

    idx_per_e = []
    w_per_e = []
    for e in range(E):
        sel = np.nonzero(top2 == e)
        idx_per_e.append(sel[0])
        w_per_e.append(topv[sel].astype(np.float32))

    affine = {
        "b1": not np.all(inp["b1"] == 0.0),
        "g1": not np.all(inp["g1"] == 1.0),
        "be1": not np.all(inp["be1"] == 0.0),
        "b2": not np.all(inp["b2"] == 0.0),
        "g2": not np.all(inp["g2"] == 1.0),
        "be2": not np.all(inp["be2"] == 0.0),
        "b3": not np.all(inp["b3"] == 0.0),
    }
    any_affine = any(affine.values())

    tiles_e = [int(math.ceil(len(ix) / P)) for ix in idx_per_e]
    if any_affine:
        # affine params are per-expert; keep one expert per core
        sA, sB = max(max(tiles_e), 1), 0
        assign = [(1, 0)] * E
    else:
        sA, sB, assign = _pack_segments(tiles_e)

    # build slot lists: each slot = (expert, first_piece, n_pieces)
    slotsA, slotsB = [], []
    for e in range(E):
        a_e, b_e = assign[e]
        pos = 0
        nt = tiles_e[e]
        for _ in range(a_e):
            take = max(0, min(sA, nt - pos))
            slotsA.append((e, pos, take))
            pos += take
        for _ in range(b_e):
            take = max(0, min(sB, nt - pos))
            slotsB.append((e, pos, take))
            pos += take
    while len(slotsA) < E:
        slotsA.append((None, 0, 0))
    while len(slotsB) < E:
        slotsB.append((None, 0, 0))

    n_tiles = sA + sB
    C = n_tiles * P
    bf = ml_dtypes.bfloat16
    CS = 512
    zW1 = np.zeros((4, P, D // P, CS), bf)
    zW1b = np.zeros((P, D // P, H), bf)
    zW2 = np.zeros((4, P, H // P, CS), bf)
    zW3b = np.zeros((P, H // P, D), bf)

    def slot_tokens(slot, s_cap):
        """token columns (D, s_cap*P) f32 + cw (s_cap*P,) for one slot."""
        e, pos, take = slot
        tt = np.zeros((D, s_cap * P), np.float32)
        cw = np.zeros((s_cap * P,), np.float32)
        if e is not None and take > 0:
            lo = pos * P
            hi = min(len(idx_per_e[e]), (pos + take) * P)
            n = hi - lo
            tt[:, :n] = t[idx_per_e[e][lo:hi]].T
            cw[:n] = w_per_e[e][lo:hi]
        return tt, cw

    def chunked(e, which):
        w = (inp["W1"], inp["W2"], inp["W3"])[which][e]
        kk = (D // P, H // P, H // P)[which]
        return np.ascontiguousarray(w).reshape(kk, P, w.shape[1]).astype(bf)

    def w1_strips(e):  # (4, P, KD, CS)
        if e is None:
            return zW1
        ch = chunked(e, 0)
        return np.ascontiguousarray(np.stack(
            [ch[:, :, c * CS:(c + 1) * CS].transpose(1, 0, 2) for c in range(4)]
        ))

    def w1_whole(e):  # (P, KD, H)
        if e is None:
            return zW1b
        return np.ascontiguousarray(chunked(e, 0).transpose(1, 0, 2))

    def w2_strips(e):  # (4, P, KH, CS)
        if e is None:
            return zW2
        ch = chunked(e, 1)
        return np.ascontiguousarray(np.stack([
            ch[:, :, c * CS:(c + 1) * CS].transpose(1, 0, 2) for c in range(4)
        ]))

    def w3_whole(e):  # (P, KH, D)
        if e is None:
            return zW3b
        return np.ascontiguousarray(chunked(e, 2).transpose(1, 0, 2))

    in_maps = []
    for c in range(E):
        ttA, cwA = slot_tokens(slotsA[c], sA)
        eA = slotsA[c][0]
        if sB > 0:
            ttB, cwB = slot_tokens(slotsB[c], sB)
            tt = np.concatenate([ttA, ttB], axis=1)
            cw = np.concatenate([cwA, cwB])
        else:
            tt, cw = ttA, cwA
        tTp = np.ascontiguousarray(
            tt.reshape(D // P, P, C).transpose(1, 0, 2)
        ).astype(bf)  # (P, KD, C)
        NF = min(6, n_tiles)
        F3 = NF * P
        m = {
            "tT0": np.ascontiguousarray(tTp[:, :, 0:P]),
            "W1a": w1_strips(eA),
            "W2a": w2_strips(eA),
            "W3a": w3_whole(eA),
            "cw": np.ascontiguousarray(cw.reshape(n_tiles, P).T).astype(np.float32),
        }
        if NF > 1:
            m["tTm"] = np.ascontiguousarray(tTp[:, :, P:F3])
        if F3 < C:
            m["tTr"] = np.ascontiguousarray(tTp[:, :, F3:C])
        if sB > 0:
            eB = slotsB[c][0]
            m["W1b"] = w1_whole(eB)
            m["W2b"] = w2_strips(eB)
            m["W3b"] = w3_whole(eB)
        for name in ("b1", "g1", "be1", "b2", "g2", "be2", "b3"):
            if affine[name]:
                row = np.asarray(inp[name][eA if eA is not None else 0], np.float32)
                m[name] = np.ascontiguousarray(np.broadcast_to(row, (P, row.shape[0])))
        in_maps.append(m)

    nc = _build_program(sA, sB, affine)

    trace = bool(os.environ.get("KERNEL_TRACE"))
    if trace:
        try:
            from antenv import axon_hooks as _ah  # noqa: F401
        except ImportError:
            trace = False
    tdir = os.environ.get("KERNEL_TRACE_DIR") or None
    try:
        res = bass_utils.run_bass_kernel_spmd(
            nc, in_maps, core_ids=list(range(E)), trace=trace, tmpdir=tdir
        )
    except Exception:
        if not trace:
            raise
        res = bass_utils.run_bass_kernel_spmd(
            nc, in_maps, core_ids=list(range(E)), trace=False
        )
    last_exec_time_ns = getattr(res, "exec_time_ns", None)
    globals()["last_result"] = res

    out_full = np.zeros((T, D), np.float32)
    for c in range(E):
        o = np.asarray(res.results[c]["out"], np.float32)
        for si, (slot, s_cap, base) in enumerate(
            ((slotsA[c], sA, 0), (slotsB[c], sB, sA * P))
        ):
            e, pos, take = slot
            if e is None or take == 0:
                continue
            lo = pos * P
            hi = min(len(idx_per_e[e]), (pos + take) * P)
            n = hi - lo
            out_full[idx_per_e[e][lo:hi]] += o[base:base + n]
    return out_full.reshape(B, S, D).astype(np.float32)


# revision 46
# speedup vs baseline: 1.2467x; 1.2467x over previous
"""MoE kernel for 8 TRN2 NeuronCores.

Strategy (expert-parallel, routing-as-sharding):
  - Router (Linear-GELU-Linear-softmax-top2) runs on host in f64 numpy;
    verified to reproduce the jax f32 reference top-2 sets exactly.
  - Token tiles (128 tokens, single expert each) are bin-packed onto the
    8 cores in up to two uniform "segments" per core: segment A runs sA
    tiles with one expert's weights, segment B runs sB tiles with a second
    expert's weights (loaded mid-kernel, overlapped with compute).
  - Per-core Bass kernel: 3-layer expert MLP with LayerNorm+exact-GELU
    between layers, bf16 matmuls with f32 PSUM accumulation, LN stats read
    PSUM directly, combine-weight scaling fused into output eviction.
    Software-pipelined across tiles (3-stage skew) to keep the PE busy.
  - LN rstd is a quake-style inverse sqrt on the vector engine (seed via
    exponent bit-hack + Newton), so the scalar engine only ever runs GELU
    and its activation table loads exactly once.
  - Weight/token DMAs are issued in consumption order (c-strips) across
    the idle queues; dummy identity matmuls warm the PE clock (HAM) while
    the first weights stream in.
  - Host scatter-adds the two expert contributions per token.
"""

import math
import os

import numpy as np

D, H, E, K = 512, 2048, 8, 2
EPS = 1e-5
P = 128
# quake rsqrt magic, pre-adjusted so the seed read from bits of hv=ve/2
# approximates ve^-1/2 (0x5f3759df - 0x00400000)
QUAKE_MAGIC = 0x5F3759DF - 0x00400000
NR_ITERS = 1

last_exec_time_ns = None


def _gelu_exact(x):
    from scipy.special import erf

    return 0.5 * x * (1.0 + erf(x / np.sqrt(2.0)))


def _route(t, Wg1, bg1, Wg2, bg2):
    th = t.astype(np.float64)
    h = th @ Wg1.astype(np.float64) + bg1.astype(np.float64)
    h = _gelu_exact(h)
    logits = h @ Wg2.astype(np.float64) + bg2.astype(np.float64)
    logits = logits - logits.max(axis=-1, keepdims=True)
    ex = np.exp(logits)
    gates = ex / ex.sum(axis=-1, keepdims=True)
    top2 = np.argsort(-gates, axis=-1, kind="stable")[:, :K]
    topv = np.take_along_axis(gates, top2, axis=-1)
    topv = topv / topv.sum(axis=-1, keepdims=True)
    return top2, topv.astype(np.float32)


def _pack_segments(tiles, n_slots=8):
    """Find minimal S and split S = sA + sB such that every expert's tile
    count can be covered by a_e A-slots (sA tiles each) + b_e B-slots (sB
    tiles each) with sum(a) <= n_slots, sum(b) <= n_slots.

    Returns (sA, sB, assign) where assign[e] = (a_e, b_e)."""
    total = sum(tiles)
    s_lo = max(1, (total + n_slots - 1) // n_slots)
    for S in range(s_lo, max(tiles) + 1):
        for sA in range(S, (S - 1) // 2, -1):
            sB = S - sA
            states = {(0, 0): 0}
            back = []
            ok = True
            for t in tiles:
                opts = []
                for a in range(n_slots + 1):
                    for b in range(n_slots + 1):
                        cap = a * sA + b * sB
                        if cap >= t:
                            opts.append((a, b, cap - t))
                new = {}
                for (au, bu), w in states.items():
                    for a, b, waste in opts:
                        if au + a <= n_slots and bu + b <= n_slots:
                            key = (au + a, bu + b)
                            val = (w + waste, (au, bu), (a, b))
                            if key not in new or new[key][0] > val[0]:
                                new[key] = val
                if not new:
                    ok = False
                    break
                back.append(new)
                states = {k: v[0] for k, v in new.items()}
            if not ok:
                continue
            key = min(states, key=lambda k: states[k])
            assign = []
            for st in reversed(back):
                w, prev, ab = st[key]
                assign.append(ab)
                key = prev
            return sA, sB, list(reversed(assign))
    return max(tiles), 0, [(1, 0)] * len(tiles)


def _build_program(n_a, n_b, affine, compute_dt_name="bfloat16"):
    """Per-core Bass program: n_a tiles with weight-set A, then n_b tiles
    with weight-set B (B weights streamed in mid-kernel)."""
    from concourse import bacc, bass, tile, mybir
    from concourse import masks

    f32 = mybir.dt.float32
    u32 = mybir.dt.uint32
    bf16 = getattr(mybir.dt, compute_dt_name)
    AF = mybir.ActivationFunctionType
    ALU = mybir.AluOpType

    n_tiles = n_a + n_b
    C = n_tiles * P
    two_seg = n_b > 0
    KD = D // P
    KH = H // P
    KQ = KH // 4
    CS = 512

    nc = bacc.Bacc(None, target_bir_lowering=False, debug=False)

    # host-packed, partition-major pieces chosen so BOTH the DRAM source and
    # the SBUF destination of every DMA are fully contiguous per partition:
    # strided SBUF writes drop the per-queue rate from ~300GB/s to ~100GB/s,
    # and descriptor generation cost scales with row count. Weight strips
    # therefore live in per-c-strip SBUF tiles.
    NF = min(6, n_tiles)  # prologue depth
    F3 = NF * P
    tT0_d = nc.dram_tensor("tT0", (P, KD, P), bf16, kind="ExternalInput")
    if NF > 1:
        tTm_d = nc.dram_tensor("tTm", (P, KD, F3 - P), bf16, kind="ExternalInput")
    if F3 < C:
        tTr_d = nc.dram_tensor("tTr", (P, KD, C - F3), bf16, kind="ExternalInput")
    w1a_d = nc.dram_tensor("W1a", (4, P, KD, CS), bf16, kind="ExternalInput")
    w2a_d = nc.dram_tensor("W2a", (4, P, KH, CS), bf16, kind="ExternalInput")
    w3a_d = nc.dram_tensor("W3a", (P, KH, D), bf16, kind="ExternalInput")
    cw_d = nc.dram_tensor("cw", (P, n_tiles), f32, kind="ExternalInput")
    out_d = nc.dram_tensor("out", (C, D), f32, kind="ExternalOutput")
    if two_seg:
        w1b_d = nc.dram_tensor("W1b", (P, KD, H), bf16, kind="ExternalInput")
        w2b_d = nc.dram_tensor("W2b", (4, P, KH, CS), bf16, kind="ExternalInput")
        w3b_d = nc.dram_tensor("W3b", (P, KH, D), bf16, kind="ExternalInput")

    aff_d = {}
    for name, width in (
        ("b1", H), ("g1", H), ("be1", H),
        ("b2", H), ("g2", H), ("be2", H),
        ("b3", D),
    ):
        if affine[name]:
            aff_d[name] = nc.dram_tensor(name, (P, width), f32, kind="ExternalInput")

    with tile.TileContext(nc) as tc:
        with (
            tc.tile_pool(name="const", bufs=1) as const_pool,
            tc.tile_pool(name="hraw", bufs=2) as hraw_pool,
            tc.tile_pool(name="xg1", bufs=7) as xg1_pool,
            tc.tile_pool(name="xg2", bufs=2) as xg2_pool,
            tc.tile_pool(name="hT", bufs=2) as hT_pool,
            tc.tile_pool(name="outp", bufs=2) as out_pool,
            tc.tile_pool(name="st", bufs=8) as st_pool,
            tc.tile_pool(name="acc", bufs=6, space="PSUM") as acc_pool,
            tc.tile_pool(name="tp", bufs=2, space="PSUM") as tp_pool,
        ):
            # ---- resident tiles (weights per c-strip for contiguous DMA) --
            w1a_c = [const_pool.tile((P, KD, CS), bf16, name=f"w1a{c}")
                     for c in range(4)]
            w2a_c = [const_pool.tile((P, KH, CS), bf16, name=f"w2a{c}")
                     for c in range(4)]
            w3a_s = const_pool.tile((P, KH, D), bf16)
            tT0_s = const_pool.tile((P, KD, P), bf16)
            if NF > 1:
                tTm_s = const_pool.tile((P, KD, F3 - P), bf16)
            if F3 < C:
                tTr_s = const_pool.tile((P, KD, C - F3), bf16)
            cw_s = const_pool.tile((P, n_tiles), f32)
            if two_seg:
                w1b_s = const_pool.tile((P, KD, H), bf16)
                w3b_s = const_pool.tile((P, KH, D), bf16)

            # ---- PE clock warm-up prep: a dep-free bf16 operand tile so
            # dummy matmuls can start the HAM un-throttle (~3.4us busy)
            # as soon as the engine preamble ends
            dummy_bf = const_pool.tile((P, P), bf16, name="dummy_bf")
            nc.vector.memset(dummy_bf[:], 1.0)
            warm_ps = acc_pool.tile((P, P), f32, name="warm_ps", tag="ps_acc")
            for _ in range(24):
                nc.tensor.matmul(warm_ps[:], dummy_bf[:], dummy_bf[:],
                                 start=True, stop=True)

            # ---- resident loads: one DMA per contiguous piece, each
            # queue's FIFO ordered by consumption deadline. The scalar
            # (Activation) engine gets only ONE issue: a dma_start whose
            # queue backpressures blocks the engine FIFO, and every GELU
            # eviction sits behind it — sync and gpsimd carry the bulk ----
            nc.scalar.dma_start(w1a_c[0][:], w1a_d[0])
            # sync: tile-0 tokens, prologue tokens, W2a even quads, cw
            nc.sync.dma_start(tT0_s[:], tT0_d[:])
            if NF > 1:
                nc.sync.dma_start(tTm_s[:], tTm_d[:])
            for c in range(4):
                for q in (0, 2):
                    nc.sync.dma_start(
                        w2a_c[c][:, 4 * q:4 * q + 4, :], w2a_d[c, :, 4 * q:4 * q + 4, :]
                    )
            nc.sync.dma_start(cw_s[:], cw_d[:])
            # gpsimd: W1a c1-c3, token rest, W2a odd quads, W3a halves
            nc.gpsimd.dma_start(w1a_c[1][:], w1a_d[1])
            nc.gpsimd.dma_start(w1a_c[2][:], w1a_d[2])
            nc.gpsimd.dma_start(w1a_c[3][:], w1a_d[3])
            if F3 < C:
                nc.gpsimd.dma_start(tTr_s[:], tTr_d[:])
            for c in range(4):
                for q in (1, 3):
                    nc.gpsimd.dma_start(
                        w2a_c[c][:, 4 * q:4 * q + 4, :], w2a_d[c, :, 4 * q:4 * q + 4, :]
                    )
            nc.gpsimd.dma_start(w3a_s[:, 0:8, :], w3a_d[:, 0:8, :])
            nc.gpsimd.dma_start(w3a_s[:, 8:16, :], w3a_d[:, 8:16, :])

            # ---- small constants (after the DMA issues so their engine
            # work does not delay descriptor generation) ----
            identity = const_pool.tile((P, P), bf16)
            masks.make_identity(nc, identity[:])
            magic_t = const_pool.tile((P, 1), u32, name="magic_t")
            nc.vector.memset(magic_t[:], QUAKE_MAGIC)
            c15_t = const_pool.tile((P, 1), f32, name="c15_t")
            nc.vector.memset(c15_t[:], 1.5)
            warm_t = const_pool.tile((P, 1), f32, name="warm_t")
            nc.vector.memset(warm_t[:], 0.0)
            # preload the GELU activation table while DMAs stream (bias/
            # scale APs match the real evictions' ACT variant)
            nc.scalar.activation(warm_t[:], warm_t[:], AF.Gelu,
                                 bias=c15_t[:], scale=c15_t[:])

            aff_s = {}
            for name in aff_d:
                width = aff_d[name].shape[1]
                row = const_pool.tile((P, width), f32, name=f"{name}_bcast")
                nc.sync.dma_start(row[:], aff_d[name][:])
                aff_s[name] = row

            def w1a_get(k, c):
                return w1a_c[c][:, k, :]

            def w1b_get(k, c):
                return w1b_s[:, k, c * CS:(c + 1) * CS]

            def w2_get(k, c):
                return w2a_c[c][:, k, :]

            def weights_for(i):
                if (not two_seg) or i < n_a:
                    return w1a_get, w2_get, w3a_s
                return w1b_get, w2_get, w3b_s

            def tok_lhsT(i, k):
                if i == 0:
                    return tT0_s[:, k, :]
                if i < NF:
                    return tTm_s[:, k, (i - 1) * P:i * P]
                return tTr_s[:, k, (i - NF) * P:(i - NF + 1) * P]

            def quake_rstd_negmr(mv):
                """rstd = (var+eps)^-1/2 and negmr = -mean*rstd, DVE only."""
                hv = st_pool.tile((P, 1), f32, name="hv", tag="hv")
                nc.vector.tensor_scalar(
                    out=hv[:], in0=mv[:, 1:2], scalar1=float(EPS),
                    scalar2=0.5, op0=ALU.add, op1=ALU.mult,
                )
                ysh = st_pool.tile((P, 1), f32, name="ysh", tag="ysh")
                nc.vector.tensor_scalar(
                    out=ysh[:].bitcast(u32), in0=hv[:].bitcast(u32),
                    scalar1=1, scalar2=None, op0=ALU.arith_shift_right,
                )
                y = st_pool.tile((P, 1), f32, name="yq", tag="yq")
                nc.vector.tensor_tensor(
                    out=y[:].bitcast(u32), in0=magic_t[:],
                    in1=ysh[:].bitcast(u32), op=ALU.subtract,
                )
                for _ in range(NR_ITERS):
                    a = st_pool.tile((P, 1), f32, name="aq", tag="aq")
                    nc.vector.tensor_tensor(
                        out=a[:], in0=y[:], in1=y[:], op=ALU.mult,
                    )
                    cq = st_pool.tile((P, 1), f32, name="cq", tag="cq")
                    nc.vector.scalar_tensor_tensor(
                        out=cq[:], in0=a[:], scalar=hv[:], in1=c15_t[:],
                        op0=ALU.mult, op1=ALU.subtract,
                    )
                    y2 = st_pool.tile((P, 1), f32, name="y2q", tag="y2q")
                    nc.vector.tensor_scalar(
                        out=y2[:], in0=cq[:], scalar1=y[:], scalar2=-1.0,
                        op0=ALU.mult, op1=ALU.mult,
                    )
                    y = y2
                negmr = st_pool.tile((P, 1), f32, name="negmr", tag="negmr")
                nc.vector.tensor_scalar(
                    out=negmr[:], in0=mv[:, 0:1], scalar1=y[:], scalar2=-1.0,
                    op0=ALU.mult, op1=ALU.mult,
                )
                return y, negmr

            def mm_ln_gelu(tile_i, lhsT_getter, n_k, w_get, nh, bname, gname, bename, xg_tag):
                """matmul (-> +b) -> LN -> (*g +be) -> gelu; returns xg tile."""
                nch = nh // CS
                fast = not (affine[bname] or affine[gname] or affine[bename])
                hraw = None
                if not fast:
                    hraw = hraw_pool.tile((P, nh), f32, tag="hraw")
                stats = st_pool.tile((P, nch, 6), f32, tag="stats")
                ps_list = []
                for c in range(nch):
                    ps = acc_pool.tile((P, CS), f32, name="ps_acc", tag="ps_acc")
                    for k in range(n_k):
                        nc.tensor.matmul(
                            ps[:],
                            lhsT_getter(k),
                            w_get(k, c),
                            start=(k == 0),
                            stop=(k == n_k - 1),
                        )
                    cs_sl = slice(c * CS, (c + 1) * CS)
                    if fast:
                        nc.vector.bn_stats(stats[:, c, :], ps[:])
                        ps_list.append(ps)
                    else:
                        nc.scalar.copy(hraw[:, cs_sl], ps[:])
                        if affine[bname]:
                            nc.vector.tensor_tensor(
                                out=hraw[:, cs_sl], in0=hraw[:, cs_sl],
                                in1=aff_s[bname][:, cs_sl], op=ALU.add,
                            )
                        nc.vector.bn_stats(stats[:, c, :], hraw[:, cs_sl])
                mv = st_pool.tile((P, 2), f32, tag="mv")
                nc.vector.bn_aggr(mv[:], stats[:])
                rstd, negmr = quake_rstd_negmr(mv)
                pool = xg1_pool if xg_tag == "xg1" else xg2_pool
                xg = pool.tile((P, nh), bf16, tag=xg_tag)
                for c in range(nch):
                    cs_sl = slice(c * CS, (c + 1) * CS)
                    if fast:
                        nc.scalar.activation(
                            xg[:, cs_sl], ps_list[c][:], AF.Gelu,
                            bias=negmr[:], scale=rstd[:],
                        )
                    else:
                        xn = hraw_pool.tile((P, CS), f32, name="xn", tag="xn")
                        nc.vector.tensor_scalar(
                            out=xn[:], in0=hraw[:, cs_sl],
                            scalar1=mv[:, 0:1], scalar2=rstd[:],
                            op0=ALU.subtract, op1=ALU.mult,
                        )
                        if affine[gname]:
                            nc.vector.tensor_tensor(
                                out=xn[:], in0=xn[:], in1=aff_s[gname][:, cs_sl],
                                op=ALU.mult,
                            )
                        if affine[bename]:
                            nc.vector.tensor_tensor(
                                out=xn[:], in0=xn[:], in1=aff_s[bename][:, cs_sl],
                                op=ALU.add,
                            )
                        nc.scalar.activation(xg[:, cs_sl], xn[:], AF.Gelu)
                return xg

            def transpose_to_hT(xg, nh, hT_tag):
                """PE-transpose (P, nh) bf16 -> (P, nh//P, P) feature-major.
                Batches 8 blocks per PSUM tile (1 full bank) so the DVE
                evacuation keeps pace with the PE transposes."""
                TB = 2 * CS  # 1024 cols = 8 blocks per psum tile
                nbt = nh // TB
                hT = hT_pool.tile((P, nh // P, P), bf16, tag=hT_tag)
                for half in range(nbt):
                    pt = tp_pool.tile((P, TB), bf16, name="pt", tag="pt")
                    for j in range(TB // P):
                        b = half * (TB // P) + j
                        nc.tensor.transpose(
                            pt[:, j * P:(j + 1) * P],
                            xg[:, b * P:(b + 1) * P],
                            identity[:],
                        )
                    nc.vector.tensor_copy(
                        hT[:, half * (TB // P):(half + 1) * (TB // P), :], pt[:]
                    )
                return hT

            xg1 = {}
            xg2 = {}

            def stage_a(i):
                w1_get = weights_for(i)[0]
                xg1[i] = mm_ln_gelu(
                    i, lambda k: tok_lhsT(i, k), KD, w1_get, H,
                    "b1", "g1", "be1", "xg1",
                )

            def stage_b(i):
                w2_get = weights_for(i)[1]
                h1T = transpose_to_hT(xg1.pop(i), H, "hT1")
                xg2[i] = mm_ln_gelu(
                    i, lambda k: h1T[:, k, :], KH, w2_get, H,
                    "b2", "g2", "be2", "xg2",
                )

            def stage_c(i):
                w3_s = weights_for(i)[2]
                h2T = transpose_to_hT(xg2.pop(i), H, "hT2")
                ps3 = acc_pool.tile((P, D), f32, name="ps3", tag="ps_acc")
                for k in range(KH):
                    nc.tensor.matmul(
                        ps3[:], h2T[:, k, :], w3_s[:, k, :],
                        start=(k == 0), stop=(k == KH - 1),
                    )
                outt = out_pool.tile((P, D), f32, tag="outt")
                if affine["b3"]:
                    nc.vector.tensor_tensor(
                        out=outt[:], in0=ps3[:], in1=aff_s["b3"][:], op=ALU.add,
                    )
                    nc.scalar.mul(outt[:], outt[:], cw_s[:, i:i + 1])
                else:
                    nc.vector.tensor_scalar(
                        out=outt[:], in0=ps3[:],
                        scalar1=cw_s[:, i:i + 1], scalar2=None,
                        op0=ALU.mult, op1=ALU.bypass,
                    )
                tok = slice(i * P, (i + 1) * P)
                nc.gpsimd.dma_start(out_d[tok, :], outt[:])

            PD = NF
            for i in range(PD):
                stage_a(i)
            for j in range(n_tiles):
                if j + PD < n_tiles:
                    stage_a(j + PD)
                # segment-B W1 rides the scalar queue at j==0: its single
                # phase-0 transfer has drained so these issues don't block
                # the GELU stream, and emission still precedes every
                # stage_a(i>=n_a) read (those start at j>=1)
                if two_seg and j == 0:
                    nc.scalar.dma_start(w1b_s[:, 0:2, :], w1b_d[:, 0:2, :])
                    nc.scalar.dma_start(w1b_s[:, 2:4, :], w1b_d[:, 2:4, :])
                stage_b(j)
                if two_seg and j == 0:
                    nc.scalar.dma_start(w3b_s[:, 0:8, :], w3b_d[:, 0:8, :])
                    nc.scalar.dma_start(w3b_s[:, 8:16, :], w3b_d[:, 8:16, :])
                if j >= 1:
                    stage_c(j - 1)
                # after the last segment-A consumer of W2 is issued,
                # stream segment-B W2 into the same SBUF tiles, c-strip
                # order, on sync+gpsimd (their FIFOs may block on the WAR
                # semaphore; scalar's must not). Emitted after stage_c so
                # the preceding out-DMA issue isn't stuck behind the WAR.
                if two_seg and j == n_a - 1:
                    for c in range(4):
                        nc.sync.dma_start(w2a_c[c][:, 0:8, :], w2b_d[c, :, 0:8, :])
                        nc.gpsimd.dma_start(w2a_c[c][:, 8:16, :], w2b_d[c, :, 8:16, :])
            stage_c(n_tiles - 1)

    nc.compile()
    return nc


def kernel(**inputs):
    global last_exec_time_ns
    import ml_dtypes

    from concourse import bass_utils

    inp = {k: np.asarray(v) for k, v in inputs.items()}
    x = inp["x"].astype(np.float32, copy=False)
    B, S, d = x.shape
    T = B * S
    t = x.reshape(T, d)

    top2, topv = _route(t, inp["Wg1"], inp["bg1"], inp["Wg2"], inp["bg2"])

    idx_per_e = []
    w_per_e = []
    for e in range(E):
        sel = np.nonzero(top2 == e)
        idx_per_e.append(sel[0])
        w_per_e.append(topv[sel].astype(np.float32))

    affine = {
        "b1": not np.all(inp["b1"] == 0.0),
        "g1": not np.all(inp["g1"] == 1.0),
        "be1": not np.all(inp["be1"] == 0.0),
        "b2": not np.all(inp["b2"] == 0.0),
        "g2": not np.all(inp["g2"] == 1.0),
        "be2": not np.all(inp["be2"] == 0.0),
        "b3": not np.all(inp["b3"] == 0.0),
    }
    any_affine = any(affine.values())

    tiles_e = [int(math.ceil(len(ix) / P)) for ix in idx_per_e]
    if any_affine:
        # affine params are per-expert; keep one expert per core
        sA, sB = max(max(tiles_e), 1), 0
        assign = [(1, 0)] * E
    else:
        sA, sB, assign = _pack_segments(tiles_e)

    # build slot lists: each slot = (expert, first_piece, n_pieces)
    slotsA, slotsB = [], []
    for e in range(E):
        a_e, b_e = assign[e]
        pos = 0
        nt = tiles_e[e]
        for _ in range(a_e):
            take = max(0, min(sA, nt - pos))
            slotsA.append((e, pos, take))
            pos += take
        for _ in range(b_e):
            take = max(0, min(sB, nt - pos))
            slotsB.append((e, pos, take))
            pos += take
    while len(slotsA) < E:
        slotsA.append((None, 0, 0))
    while len(slotsB) < E:
        slotsB.append((None, 0, 0))

    n_tiles = sA + sB
    C = n_tiles * P
    bf = ml_dtypes.bfloat16
    CS = 512
    zW1 = np.zeros((4, P, D // P, CS), bf)
    zW1b = np.zeros((P, D // P, H), bf)
    zW2 = np.zeros((4, P, H // P, CS), bf)
    zW3b = np.zeros((P, H // P, D), bf)

    def slot_tokens(slot, s_cap):
        """token columns (D, s_cap*P) f32 + cw (s_cap*P,) for one slot."""
        e, pos, take = slot
        tt = np.zeros((D, s_cap * P), np.float32)
        cw = np.zeros((s_cap * P,), np.float32)
        if e is not None and take > 0:
            lo = pos * P
            hi = min(len(idx_per_e[e]), (pos + take) * P)
            n = hi - lo
            tt[:, :n] = t[idx_per_e[e][lo:hi]].T
            cw[:n] = w_per_e[e][lo:hi]
        return tt, cw

    def chunked(e, which):
        w = (inp["W1"], inp["W2"], inp["W3"])[which][e]
        kk = (D // P, H // P, H // P)[which]
        return np.ascontiguousarray(w).reshape(kk, P, w.shape[1]).astype(bf)

    def w1_strips(e):  # (4, P, KD, CS)
        if e is None:
            return zW1
        ch = chunked(e, 0)
        return np.ascontiguousarray(np.stack(
            [ch[:, :, c * CS:(c + 1) * CS].transpose(1, 0, 2) for c in range(4)]
        ))

    def w1_whole(e):  # (P, KD, H)
        if e is None:
            return zW1b
        return np.ascontiguousarray(chunked(e, 0).transpose(1, 0, 2))

    def w2_strips(e):  # (4, P, KH, CS)
        if e is None:
            return zW2
        ch = chunked(e, 1)
        return np.ascontiguousarray(np.stack([
            ch[:, :, c * CS:(c + 1) * CS].transpose(1, 0, 2) for c in range(4)
        ]))

    def w3_whole(e):  # (P, KH, D)
        if e is None:
            return zW3b
        return np.ascontiguousarray(chunked(e, 2).transpose(1, 0, 2))

    in_maps = []
    for c in range(E):
        ttA, cwA = slot_tokens(slotsA[c], sA)
        eA = slotsA[c][0]
        if sB > 0:
            ttB, cwB = slot_tokens(slotsB[c], sB)
            tt = np.concatenate([ttA, ttB], axis=1)
            cw = np.concatenate([cwA, cwB])
        else:
            tt, cw = ttA, cwA
        tTp = np.ascontiguousarray(
            tt.reshape(D // P, P, C).transpose(1, 0, 2)
        ).astype(bf)  # (P, KD, C)
        NF = min(6, n_tiles)
        F3 = NF * P
        m = {
            "tT0": np.ascontiguousarray(tTp[:, :, 0:P]),
            "W1a": w1_strips(eA),
            "W2a": w2_strips(eA),
            "W3a": w3_whole(eA),
            "cw": np.ascontiguousarray(cw.reshape(n_tiles, P).T).astype(np.float32),
        }
        if NF > 1:
            m["tTm"] = np.ascontiguousarray(tTp[:, :, P:F3])
        if F3 < C:
            m["tTr"] = np.ascontiguousarray(tTp[:, :, F3:C])
        if sB > 0:
            eB = slotsB[c][0]
            m["W1b"] = w1_whole(eB)
            m["W2b"] = w2_strips(eB)
            m["W3b"] = w3_whole(eB)
        for name in ("b1", "g1", "be1", "b2", "g2", "be2", "b3"):
            if affine[name]:
                row = np.asarray(inp[name][eA if eA is not None else 0], np.float32)
                m[name] = np.ascontiguousarray(np.broadcast_to(row, (P, row.shape[0])))
        in_maps.append(m)

    nc = _build_program(sA, sB, affine)

    trace = bool(os.environ.get("KERNEL_TRACE"))
    if trace:
        try:
            from antenv import axon_hooks as _ah  # noqa: F401
        except ImportError:
            trace = False
    tdir = os.environ.get("KERNEL_TRACE_DIR") or None
    try:
        res = bass_utils.run_bass_kernel_spmd(
            nc, in_maps, core_ids=list(range(E)), trace=trace, tmpdir=tdir
        )
    except Exception:
        if not trace:
            raise
        res = bass_utils.run_bass_kernel_spmd(
            nc, in_maps, core_ids=list(range(E)), trace=False
        )
    last_exec_time_ns = getattr(res, "exec_time_ns", None)
    globals()["last_result"] = res

    out_full = np.zeros((T, D), np.float32)
    for c in range(E):
        o = np.asarray(res.results[c]["out"], np.float32)
        for si, (slot, s_cap, base) in enumerate(
            ((slotsA[c], sA, 0), (slotsB[c], sB, sA * P))
        ):
            e, pos, take = slot
            if e is None or take == 0:
                continue
            lo = pos * P
            hi = min(len(idx_per_e[e]), (pos + take) * P)
            n = hi - lo
            out_full[idx_per_e[e][lo:hi]] += o[base:base + n]
    return out_full.reshape(B, S, D).astype(np.float32)


# revision 47
# speedup vs baseline: 1.3013x; 1.0438x over previous
"""MoE kernel for 8 TRN2 NeuronCores.

Strategy (expert-parallel, routing-as-sharding):
  - Router (Linear-GELU-Linear-softmax-top2) runs on host in f64 numpy;
    verified to reproduce the jax f32 reference top-2 sets exactly.
  - Token tiles (128 tokens, single expert each) are bin-packed onto the
    8 cores in up to two uniform "segments" per core: segment A runs sA
    tiles with one expert's weights, segment B runs sB tiles with a second
    expert's weights (loaded mid-kernel, overlapped with compute).
  - Per-core Bass kernel: 3-layer expert MLP with LayerNorm+exact-GELU
    between layers, bf16 matmuls with f32 PSUM accumulation, LN stats read
    PSUM directly, combine-weight scaling fused into output eviction.
    Software-pipelined across tiles (3-stage skew) to keep the PE busy.
  - LN rstd is a quake-style inverse sqrt on the vector engine (seed via
    exponent bit-hack + Newton), so the scalar engine only ever runs GELU
    and its activation table loads exactly once.
  - Weight/token DMAs are issued in consumption order (c-strips) across
    the idle queues; dummy identity matmuls warm the PE clock (HAM) while
    the first weights stream in.
  - Host scatter-adds the two expert contributions per token.
"""

import math
import os

import numpy as np

D, H, E, K = 512, 2048, 8, 2
EPS = 1e-5
P = 128
# quake rsqrt magic, pre-adjusted so the seed read from bits of hv=ve/2
# approximates ve^-1/2 (0x5f3759df - 0x00400000)
QUAKE_MAGIC = 0x5F3759DF - 0x00400000
NR_ITERS = 1

last_exec_time_ns = None


def _gelu_exact(x):
    from scipy.special import erf

    return 0.5 * x * (1.0 + erf(x / np.sqrt(2.0)))


def _route(t, Wg1, bg1, Wg2, bg2):
    th = t.astype(np.float64)
    h = th @ Wg1.astype(np.float64) + bg1.astype(np.float64)
    h = _gelu_exact(h)
    logits = h @ Wg2.astype(np.float64) + bg2.astype(np.float64)
    logits = logits - logits.max(axis=-1, keepdims=True)
    ex = np.exp(logits)
    gates = ex / ex.sum(axis=-1, keepdims=True)
    top2 = np.argsort(-gates, axis=-1, kind="stable")[:, :K]
    topv = np.take_along_axis(gates, top2, axis=-1)
    topv = topv / topv.sum(axis=-1, keepdims=True)
    return top2, topv.astype(np.float32)


def _pack_segments(tiles, n_slots=8):
    """Find minimal S and split S = sA + sB such that every expert's tile
    count can be covered by a_e A-slots (sA tiles each) + b_e B-slots (sB
    tiles each) with sum(a) <= n_slots, sum(b) <= n_slots.

    Returns (sA, sB, assign) where assign[e] = (a_e, b_e)."""
    total = sum(tiles)
    s_lo = max(1, (total + n_slots - 1) // n_slots)
    for S in range(s_lo, max(tiles) + 1):
        for sA in range(S, (S - 1) // 2, -1):
            sB = S - sA
            states = {(0, 0): 0}
            back = []
            ok = True
            for t in tiles:
                opts = []
                for a in range(n_slots + 1):
                    for b in range(n_slots + 1):
                        cap = a * sA + b * sB
                        if cap >= t:
                            opts.append((a, b, cap - t))
                new = {}
                for (au, bu), w in states.items():
                    for a, b, waste in opts:
                        if au + a <= n_slots and bu + b <= n_slots:
                            key = (au + a, bu + b)
                            val = (w + waste, (au, bu), (a, b))
                            if key not in new or new[key][0] > val[0]:
                                new[key] = val
                if not new:
                    ok = False
                    break
                back.append(new)
                states = {k: v[0] for k, v in new.items()}
            if not ok:
                continue
            key = min(states, key=lambda k: states[k])
            assign = []
            for st in reversed(back):
                w, prev, ab = st[key]
                assign.append(ab)
                key = prev
            return sA, sB, list(reversed(assign))
    return max(tiles), 0, [(1, 0)] * len(tiles)


def _build_program(n_a, n_b, affine, compute_dt_name="bfloat16"):
    """Per-core Bass program: n_a tiles with weight-set A, then n_b tiles
    with weight-set B (B weights streamed in mid-kernel)."""
    from concourse import bacc, bass, tile, mybir
    from concourse import masks

    f32 = mybir.dt.float32
    u32 = mybir.dt.uint32
    bf16 = getattr(mybir.dt, compute_dt_name)
    AF = mybir.ActivationFunctionType
    ALU = mybir.AluOpType

    n_tiles = n_a + n_b
    C = n_tiles * P
    two_seg = n_b > 0
    KD = D // P
    KH = H // P
    KQ = KH // 4
    CS = 512

    nc = bacc.Bacc(None, target_bir_lowering=False, debug=False)

    # host-packed, partition-major pieces chosen so BOTH the DRAM source and
    # the SBUF destination of every DMA are fully contiguous per partition:
    # strided SBUF writes drop the per-queue rate from ~300GB/s to ~100GB/s,
    # and descriptor generation cost scales with row count. Weight strips
    # therefore live in per-c-strip SBUF tiles.
    NF = min(6, n_tiles)  # prologue depth
    F3 = NF * P
    tT0_d = nc.dram_tensor("tT0", (P, KD, P), bf16, kind="ExternalInput")
    if NF > 1:
        tTm_d = nc.dram_tensor("tTm", (P, KD, F3 - P), bf16, kind="ExternalInput")
    if F3 < C:
        tTr_d = nc.dram_tensor("tTr", (P, KD, C - F3), bf16, kind="ExternalInput")
    w1a_d = nc.dram_tensor("W1a", (4, P, KD, CS), bf16, kind="ExternalInput")
    w2a_d = nc.dram_tensor("W2a", (4, P, KH, CS), bf16, kind="ExternalInput")
    w3a_d = nc.dram_tensor("W3a", (P, KH, D), bf16, kind="ExternalInput")
    cw_d = nc.dram_tensor("cw", (P, n_tiles), f32, kind="ExternalInput")
    out_d = nc.dram_tensor("out", (C, D), f32, kind="ExternalOutput")
    if two_seg:
        w1b_d = nc.dram_tensor("W1b", (P, KD, H), bf16, kind="ExternalInput")
        w2b_d = nc.dram_tensor("W2b", (4, P, KH, CS), bf16, kind="ExternalInput")
        w3b_d = nc.dram_tensor("W3b", (P, KH, D), bf16, kind="ExternalInput")

    aff_d = {}
    for name, width in (
        ("b1", H), ("g1", H), ("be1", H),
        ("b2", H), ("g2", H), ("be2", H),
        ("b3", D),
    ):
        if affine[name]:
            aff_d[name] = nc.dram_tensor(name, (P, width), f32, kind="ExternalInput")

    with tile.TileContext(nc) as tc:
        with (
            tc.tile_pool(name="const", bufs=1) as const_pool,
            tc.tile_pool(name="hraw", bufs=2) as hraw_pool,
            tc.tile_pool(name="xg1", bufs=7) as xg1_pool,
            tc.tile_pool(name="xg2", bufs=2) as xg2_pool,
            tc.tile_pool(name="hT", bufs=2) as hT_pool,
            tc.tile_pool(name="outp", bufs=2) as out_pool,
            tc.tile_pool(name="st", bufs=8) as st_pool,
            tc.tile_pool(name="acc", bufs=6, space="PSUM") as acc_pool,
            tc.tile_pool(name="tp", bufs=2, space="PSUM") as tp_pool,
        ):
            # ---- resident tiles (weights per c-strip for contiguous DMA) --
            w1a_c = [const_pool.tile((P, KD, CS), bf16, name=f"w1a{c}")
                     for c in range(4)]
            w2a_c = [const_pool.tile((P, KH, CS), bf16, name=f"w2a{c}")
                     for c in range(4)]
            w3a_s = const_pool.tile((P, KH, D), bf16)
            tT0_s = const_pool.tile((P, KD, P), bf16)
            if NF > 1:
                tTm_s = const_pool.tile((P, KD, F3 - P), bf16)
            if F3 < C:
                tTr_s = const_pool.tile((P, KD, C - F3), bf16)
            cw_s = const_pool.tile((P, n_tiles), f32)
            if two_seg:
                w1b_s = const_pool.tile((P, KD, H), bf16)
                w3b_s = const_pool.tile((P, KH, D), bf16)

            # ---- PE clock warm-up prep: a dep-free bf16 operand tile so
            # dummy matmuls can start the HAM un-throttle (~3.4us busy)
            # as soon as the engine preamble ends
            dummy_bf = const_pool.tile((P, P), bf16, name="dummy_bf")
            nc.vector.memset(dummy_bf[:], 1.0)
            warm_ps = acc_pool.tile((P, P), f32, name="warm_ps", tag="ps_acc")
            for _ in range(24):
                nc.tensor.matmul(warm_ps[:], dummy_bf[:], dummy_bf[:],
                                 start=True, stop=True)

            # ---- resident loads: one DMA per contiguous piece, each
            # queue's FIFO ordered by consumption deadline. The scalar
            # (Activation) engine gets only ONE issue: a dma_start whose
            # queue backpressures blocks the engine FIFO, and every GELU
            # eviction sits behind it — sync and gpsimd carry the bulk ----
            # scalar gets exactly two early pieces (~1MB drains before the
            # first GELU; more would block the ACT FIFO on queue space)
            nc.scalar.dma_start(w1a_c[0][:], w1a_d[0])
            nc.scalar.dma_start(w1a_c[3][:], w1a_d[3])
            # sync: tile-0 tokens, W1a c1, prologue tokens, W2a 0/1 quads
            nc.sync.dma_start(tT0_s[:], tT0_d[:])
            nc.sync.dma_start(w1a_c[1][:], w1a_d[1])
            if NF > 1:
                nc.sync.dma_start(tTm_s[:], tTm_d[:])
            for c in range(4):
                for q in (0, 1):
                    nc.sync.dma_start(
                        w2a_c[c][:, 4 * q:4 * q + 4, :], w2a_d[c, :, 4 * q:4 * q + 4, :]
                    )
            nc.sync.dma_start(cw_s[:], cw_d[:])
            # gpsimd: W1a c2, token rest, W2a 2/3 quads, W3a halves
            nc.gpsimd.dma_start(w1a_c[2][:], w1a_d[2])
            if F3 < C:
                nc.gpsimd.dma_start(tTr_s[:], tTr_d[:])
            for c in range(4):
                for q in (2, 3):
                    nc.gpsimd.dma_start(
                        w2a_c[c][:, 4 * q:4 * q + 4, :], w2a_d[c, :, 4 * q:4 * q + 4, :]
                    )
            nc.gpsimd.dma_start(w3a_s[:, 0:8, :], w3a_d[:, 0:8, :])
            nc.gpsimd.dma_start(w3a_s[:, 8:16, :], w3a_d[:, 8:16, :])

            # ---- small constants (after the DMA issues so their engine
            # work does not delay descriptor generation) ----
            identity = const_pool.tile((P, P), bf16)
            masks.make_identity(nc, identity[:])
            magic_t = const_pool.tile((P, 1), u32, name="magic_t")
            nc.vector.memset(magic_t[:], QUAKE_MAGIC)
            c15_t = const_pool.tile((P, 1), f32, name="c15_t")
            nc.vector.memset(c15_t[:], 1.5)
            warm_t = const_pool.tile((P, 1), f32, name="warm_t")
            nc.vector.memset(warm_t[:], 0.0)
            # preload the GELU activation table while DMAs stream (bias/
            # scale APs match the real evictions' ACT variant)
            nc.scalar.activation(warm_t[:], warm_t[:], AF.Gelu,
                                 bias=c15_t[:], scale=c15_t[:])

            aff_s = {}
            for name in aff_d:
                width = aff_d[name].shape[1]
                row = const_pool.tile((P, width), f32, name=f"{name}_bcast")
                nc.sync.dma_start(row[:], aff_d[name][:])
                aff_s[name] = row

            def w1a_get(k, c):
                return w1a_c[c][:, k, :]

            def w1b_get(k, c):
                return w1b_s[:, k, c * CS:(c + 1) * CS]

            def w2_get(k, c):
                return w2a_c[c][:, k, :]

            def weights_for(i):
                if (not two_seg) or i < n_a:
                    return w1a_get, w2_get, w3a_s
                return w1b_get, w2_get, w3b_s

            def tok_lhsT(i, k):
                if i == 0:
                    return tT0_s[:, k, :]
                if i < NF:
                    return tTm_s[:, k, (i - 1) * P:i * P]
                return tTr_s[:, k, (i - NF) * P:(i - NF + 1) * P]

            def quake_rstd_negmr(mv):
                """rstd = (var+eps)^-1/2 and negmr = -mean*rstd, DVE only."""
                hv = st_pool.tile((P, 1), f32, name="hv", tag="hv")
                nc.vector.tensor_scalar(
                    out=hv[:], in0=mv[:, 1:2], scalar1=float(EPS),
                    scalar2=0.5, op0=ALU.add, op1=ALU.mult,
                )
                ysh = st_pool.tile((P, 1), f32, name="ysh", tag="ysh")
                nc.vector.tensor_scalar(
                    out=ysh[:].bitcast(u32), in0=hv[:].bitcast(u32),
                    scalar1=1, scalar2=None, op0=ALU.arith_shift_right,
                )
                y = st_pool.tile((P, 1), f32, name="yq", tag="yq")
                nc.vector.tensor_tensor(
                    out=y[:].bitcast(u32), in0=magic_t[:],
                    in1=ysh[:].bitcast(u32), op=ALU.subtract,
                )
                for _ in range(NR_ITERS):
                    a = st_pool.tile((P, 1), f32, name="aq", tag="aq")
                    nc.vector.tensor_tensor(
                        out=a[:], in0=y[:], in1=y[:], op=ALU.mult,
                    )
                    cq = st_pool.tile((P, 1), f32, name="cq", tag="cq")
                    nc.vector.scalar_tensor_tensor(
                        out=cq[:], in0=a[:], scalar=hv[:], in1=c15_t[:],
                        op0=ALU.mult, op1=ALU.subtract,
                    )
                    y2 = st_pool.tile((P, 1), f32, name="y2q", tag="y2q")
                    nc.vector.tensor_scalar(
                        out=y2[:], in0=cq[:], scalar1=y[:], scalar2=-1.0,
                        op0=ALU.mult, op1=ALU.mult,
                    )
                    y = y2
                negmr = st_pool.tile((P, 1), f32, name="negmr", tag="negmr")
                nc.vector.tensor_scalar(
                    out=negmr[:], in0=mv[:, 0:1], scalar1=y[:], scalar2=-1.0,
                    op0=ALU.mult, op1=ALU.mult,
                )
                return y, negmr

            def mm_ln_gelu(tile_i, lhsT_getter, n_k, w_get, nh, bname, gname, bename, xg_tag):
                """matmul (-> +b) -> LN -> (*g +be) -> gelu; returns xg tile."""
                nch = nh // CS
                fast = not (affine[bname] or affine[gname] or affine[bename])
                hraw = None
                if not fast:
                    hraw = hraw_pool.tile((P, nh), f32, tag="hraw")
                stats = st_pool.tile((P, nch, 6), f32, tag="stats")
                ps_list = []
                for c in range(nch):
                    ps = acc_pool.tile((P, CS), f32, name="ps_acc", tag="ps_acc")
                    for k in range(n_k):
                        nc.tensor.matmul(
                            ps[:],
                            lhsT_getter(k),
                            w_get(k, c),
                            start=(k == 0),
                            stop=(k == n_k - 1),
                        )
                    cs_sl = slice(c * CS, (c + 1) * CS)
                    if fast:
                        nc.vector.bn_stats(stats[:, c, :], ps[:])
                        ps_list.append(ps)
                    else:
                        nc.scalar.copy(hraw[:, cs_sl], ps[:])
                        if affine[bname]:
                            nc.vector.tensor_tensor(
                                out=hraw[:, cs_sl], in0=hraw[:, cs_sl],
                                in1=aff_s[bname][:, cs_sl], op=ALU.add,
                            )
                        nc.vector.bn_stats(stats[:, c, :], hraw[:, cs_sl])
                mv = st_pool.tile((P, 2), f32, tag="mv")
                nc.vector.bn_aggr(mv[:], stats[:])
                rstd, negmr = quake_rstd_negmr(mv)
                pool = xg1_pool if xg_tag == "xg1" else xg2_pool
                xg = pool.tile((P, nh), bf16, tag=xg_tag)
                for c in range(nch):
                    cs_sl = slice(c * CS, (c + 1) * CS)
                    if fast:
                        nc.scalar.activation(
                            xg[:, cs_sl], ps_list[c][:], AF.Gelu,
                            bias=negmr[:], scale=rstd[:],
                        )
                    else:
                        xn = hraw_pool.tile((P, CS), f32, name="xn", tag="xn")
                        nc.vector.tensor_scalar(
                            out=xn[:], in0=hraw[:, cs_sl],
                            scalar1=mv[:, 0:1], scalar2=rstd[:],
                            op0=ALU.subtract, op1=ALU.mult,
                        )
                        if affine[gname]:
                            nc.vector.tensor_tensor(
                                out=xn[:], in0=xn[:], in1=aff_s[gname][:, cs_sl],
                                op=ALU.mult,
                            )
                        if affine[bename]:
                            nc.vector.tensor_tensor(
                                out=xn[:], in0=xn[:], in1=aff_s[bename][:, cs_sl],
                                op=ALU.add,
                            )
                        nc.scalar.activation(xg[:, cs_sl], xn[:], AF.Gelu)
                return xg

            def transpose_to_hT(xg, nh, hT_tag):
                """PE-transpose (P, nh) bf16 -> (P, nh//P, P) feature-major.
                Batches 8 blocks per PSUM tile (1 full bank) so the DVE
                evacuation keeps pace with the PE transposes."""
                TB = 2 * CS  # 1024 cols = 8 blocks per psum tile
                nbt = nh // TB
                hT = hT_pool.tile((P, nh // P, P), bf16, tag=hT_tag)
                for half in range(nbt):
                    pt = tp_pool.tile((P, TB), bf16, name="pt", tag="pt")
                    for j in range(TB // P):
                        b = half * (TB // P) + j
                        nc.tensor.transpose(
                            pt[:, j * P:(j + 1) * P],
                            xg[:, b * P:(b + 1) * P],
                            identity[:],
                        )
                    nc.vector.tensor_copy(
                        hT[:, half * (TB // P):(half + 1) * (TB // P), :], pt[:]
                    )
                return hT

            xg1 = {}
            xg2 = {}

            def stage_a(i):
                w1_get = weights_for(i)[0]
                xg1[i] = mm_ln_gelu(
                    i, lambda k: tok_lhsT(i, k), KD, w1_get, H,
                    "b1", "g1", "be1", "xg1",
                )

            def stage_b(i):
                w2_get = weights_for(i)[1]
                h1T = transpose_to_hT(xg1.pop(i), H, "hT1")
                xg2[i] = mm_ln_gelu(
                    i, lambda k: h1T[:, k, :], KH, w2_get, H,
                    "b2", "g2", "be2", "xg2",
                )

            def stage_c(i):
                w3_s = weights_for(i)[2]
                h2T = transpose_to_hT(xg2.pop(i), H, "hT2")
                ps3 = acc_pool.tile((P, D), f32, name="ps3", tag="ps_acc")
                for k in range(KH):
                    nc.tensor.matmul(
                        ps3[:], h2T[:, k, :], w3_s[:, k, :],
                        start=(k == 0), stop=(k == KH - 1),
                    )
                outt = out_pool.tile((P, D), f32, tag="outt")
                if affine["b3"]:
                    nc.vector.tensor_tensor(
                        out=outt[:], in0=ps3[:], in1=aff_s["b3"][:], op=ALU.add,
                    )
                    nc.scalar.mul(outt[:], outt[:], cw_s[:, i:i + 1])
                else:
                    nc.vector.tensor_scalar(
                        out=outt[:], in0=ps3[:],
                        scalar1=cw_s[:, i:i + 1], scalar2=None,
                        op0=ALU.mult, op1=ALU.bypass,
                    )
                tok = slice(i * P, (i + 1) * P)
                nc.gpsimd.dma_start(out_d[tok, :], outt[:])

            PD = NF
            for i in range(PD):
                stage_a(i)
            for j in range(n_tiles):
                if j + PD < n_tiles:
                    stage_a(j + PD)
                # segment-B W1 rides the scalar queue at j==0: its single
                # phase-0 transfer has drained so these issues don't block
                # the GELU stream, and emission still precedes every
                # stage_a(i>=n_a) read (those start at j>=1)
                if two_seg and j == 0:
                    nc.scalar.dma_start(w1b_s[:, 0:2, :], w1b_d[:, 0:2, :])
                    nc.scalar.dma_start(w1b_s[:, 2:4, :], w1b_d[:, 2:4, :])
                stage_b(j)
                if two_seg and j == 0:
                    nc.scalar.dma_start(w3b_s[:, 0:8, :], w3b_d[:, 0:8, :])
                    nc.scalar.dma_start(w3b_s[:, 8:16, :], w3b_d[:, 8:16, :])
                if j >= 1:
                    stage_c(j - 1)
                # after the last segment-A consumer of W2 is issued,
                # stream segment-B W2 into the same SBUF tiles, c-strip
                # order, on sync+gpsimd (their FIFOs may block on the WAR
                # semaphore; scalar's must not). Emitted after stage_c so
                # the preceding out-DMA issue isn't stuck behind the WAR.
                if two_seg and j == n_a - 1:
                    for c in range(4):
                        nc.sync.dma_start(w2a_c[c][:, 0:8, :], w2b_d[c, :, 0:8, :])
                        nc.gpsimd.dma_start(w2a_c[c][:, 8:16, :], w2b_d[c, :, 8:16, :])
            stage_c(n_tiles - 1)

    nc.compile()
    return nc


def kernel(**inputs):
    global last_exec_time_ns
    import ml_dtypes

    from concourse import bass_utils

    inp = {k: np.asarray(v) for k, v in inputs.items()}
    x = inp["x"].astype(np.float32, copy=False)
    B, S, d = x.shape
    T = B * S
    t = x.reshape(T, d)

    top2, topv = _route(t, inp["Wg1"], inp["bg1"], inp["Wg2"], inp["bg2"])

    idx_per_e = []
    w_per_e = []
    for e in range(E):
        sel = np.nonzero(top2 == e)
        idx_per_e.append(sel[0])
        w_per_e.append(topv[sel].astype(np.float32))

    affine = {
        "b1": not np.all(inp["b1"] == 0.0),
        "g1": not np.all(inp["g1"] == 1.0),
        "be1": not np.all(inp["be1"] == 0.0),
        "b2": not np.all(inp["b2"] == 0.0),
        "g2": not np.all(inp["g2"] == 1.0),
        "be2": not np.all(inp["be2"] == 0.0),
        "b3": not np.all(inp["b3"] == 0.0),
    }
    any_affine = any(affine.values())

    tiles_e = [int(math.ceil(len(ix) / P)) for ix in idx_per_e]
    if any_affine:
        # affine params are per-expert; keep one expert per core
        sA, sB = max(max(tiles_e), 1), 0
        assign = [(1, 0)] * E
    else:
        sA, sB, assign = _pack_segments(tiles_e)

    # build slot lists: each slot = (expert, first_piece, n_pieces)
    slotsA, slotsB = [], []
    for e in range(E):
        a_e, b_e = assign[e]
        pos = 0
        nt = tiles_e[e]
        for _ in range(a_e):
            take = max(0, min(sA, nt - pos))
            slotsA.append((e, pos, take))
            pos += take
        for _ in range(b_e):
            take = max(0, min(sB, nt - pos))
            slotsB.append((e, pos, take))
            pos += take
    while len(slotsA) < E:
        slotsA.append((None, 0, 0))
    while len(slotsB) < E:
        slotsB.append((None, 0, 0))

    n_tiles = sA + sB
    C = n_tiles * P
    bf = ml_dtypes.bfloat16
    CS = 512
    zW1 = np.zeros((4, P, D // P, CS), bf)
    zW1b = np.zeros((P, D // P, H), bf)
    zW2 = np.zeros((4, P, H // P, CS), bf)
    zW3b = np.zeros((P, H // P, D), bf)

    def slot_tokens(slot, s_cap):
        """token columns (D, s_cap*P) f32 + cw (s_cap*P,) for one slot."""
        e, pos, take = slot
        tt = np.zeros((D, s_cap * P), np.float32)
        cw = np.zeros((s_cap * P,), np.float32)
        if e is not None and take > 0:
            lo = pos * P
            hi = min(len(idx_per_e[e]), (pos + take) * P)
            n = hi - lo
            tt[:, :n] = t[idx_per_e[e][lo:hi]].T
            cw[:n] = w_per_e[e][lo:hi]
        return tt, cw

    def chunked(e, which):
        w = (inp["W1"], inp["W2"], inp["W3"])[which][e]
        kk = (D // P, H // P, H // P)[which]
        return np.ascontiguousarray(w).reshape(kk, P, w.shape[1]).astype(bf)

    def w1_strips(e):  # (4, P, KD, CS)
        if e is None:
            return zW1
        ch = chunked(e, 0)
        return np.ascontiguousarray(np.stack(
            [ch[:, :, c * CS:(c + 1) * CS].transpose(1, 0, 2) for c in range(4)]
        ))

    def w1_whole(e):  # (P, KD, H)
        if e is None:
            return zW1b
        return np.ascontiguousarray(chunked(e, 0).transpose(1, 0, 2))

    def w2_strips(e):  # (4, P, KH, CS)
        if e is None:
            return zW2
        ch = chunked(e, 1)
        return np.ascontiguousarray(np.stack([
            ch[:, :, c * CS:(c + 1) * CS].transpose(1, 0, 2) for c in range(4)
        ]))

    def w3_whole(e):  # (P, KH, D)
        if e is None:
            return zW3b
        return np.ascontiguousarray(chunked(e, 2).transpose(1, 0, 2))

    in_maps = []
    for c in range(E):
        ttA, cwA = slot_tokens(slotsA[c], sA)
        eA = slotsA[c][0]
        if sB > 0:
            ttB, cwB = slot_tokens(slotsB[c], sB)
            tt = np.concatenate([ttA, ttB], axis=1)
            cw = np.concatenate([cwA, cwB])
        else:
            tt, cw = ttA, cwA
        tTp = np.ascontiguousarray(
            tt.reshape(D // P, P, C).transpose(1, 0, 2)
        ).astype(bf)  # (P, KD, C)
        NF = min(6, n_tiles)
        F3 = NF * P
        m = {
            "tT0": np.ascontiguousarray(tTp[:, :, 0:P]),
            "W1a": w1_strips(eA),
            "W2a": w2_strips(eA),
            "W3a": w3_whole(eA),
            "cw": np.ascontiguousarray(cw.reshape(n_tiles, P).T).astype(np.float32),
        }
        if NF > 1:
            m["tTm"] = np.ascontiguousarray(tTp[:, :, P:F3])
        if F3 < C:
            m["tTr"] = np.ascontiguousarray(tTp[:, :, F3:C])
        if sB > 0:
            eB = slotsB[c][0]
            m["W1b"] = w1_whole(eB)
            m["W2b"] = w2_strips(eB)
            m["W3b"] = w3_whole(eB)
        for name in ("b1", "g1", "be1", "b2", "g2", "be2", "b3"):
            if affine[name]:
                row = np.asarray(inp[name][eA if eA is not None else 0], np.float32)
                m[name] = np.ascontiguousarray(np.broadcast_to(row, (P, row.shape[0])))
        in_maps.append(m)

    nc = _build_program(sA, sB, affine)

    trace = bool(os.environ.get("KERNEL_TRACE"))
    if trace:
        try:
            from antenv import axon_hooks as _ah  # noqa: F401
        except ImportError:
            trace = False
    tdir = os.environ.get("KERNEL_TRACE_DIR") or None
    try:
        res = bass_utils.run_bass_kernel_spmd(
            nc, in_maps, core_ids=list(range(E)), trace=trace, tmpdir=tdir
        )
    except Exception:
        if not trace:
            raise
        res = bass_utils.run_bass_kernel_spmd(
            nc, in_maps, core_ids=list(range(E)), trace=False
        )
    last_exec_time_ns = getattr(res, "exec_time_ns", None)
    globals()["last_result"] = res

    out_full = np.zeros((T, D), np.float32)
    for c in range(E):
        o = np.asarray(res.results[c]["out"], np.float32)
        for si, (slot, s_cap, base) in enumerate(
            ((slotsA[c], sA, 0), (slotsB[c], sB, sA * P))
        ):
            e, pos, take = slot
            if e is None or take == 0:
                continue
            lo = pos * P
            hi = min(len(idx_per_e[e]), (pos + take) * P)
            n = hi - lo
            out_full[idx_per_e[e][lo:hi]] += o[base:base + n]
    return out_full.reshape(B, S, D).astype(np.float32)
